# revision 6
# baseline (speedup 1.0000x reference)
"""Trainium2 Bass kernel for nn_GATLayer_58291296141986.

Math: the reference computes
    xt = (x @ W.T).reshape(B, N, H, D)            # B=32, N=10, H=8, D=8
    out[b,n,h,m] = relu(sum_k xt[b,n,h,k] * adj[b,n,m])
adj does not depend on k, so sum_k xt[b,n,h,k] = x[b,n,:] @ Wsum[h,:]
with Wsum[h] = sum_d W[h*8+d].  The whole problem collapses to
    s = x2 @ Wsum.T        # (320, 65536) @ (65536, 8)
    out[t, h*10+m] = relu(s[t,h] * adj[t,m])
which is memory-bound on reading x (84MB) + W (17MB).

Sharding: tensor-parallel over in_dim (k).  Each of the 8 cores reads a
disjoint 8192-wide k-slice of x (10.5MB) and W (2MB) and accumulates a
partial s64^T = W_slice @ x_slice^T of shape (64, 320) -- every input
byte is read exactly once across the chip (~12.6MB/core, the memory
roofline).  The d-reduction (rows h*8+d -> head h) and the cross-core
reduction both fold into a single ones-matmul in a second, tiny SPMD
launch: the host hands core h the 64 rows {core c, head h, d} (pure data
movement), the ones-matmul sums all 64 and replicates the result onto 10
PSUM partitions, and one fused scalar_tensor_tensor computes
relu(s)*adj^T (valid since adj >= 0, so relu(s*adj) = relu(s)*adj).
Core h thus produces the 10 output columns of head h for all 320 tokens
and the host concatenates the 8 head slices.  (A single-launch variant
with an on-device AllToAll was measured ~30us slower: the collectives
firmware's entry barrier alone costs ~60us on this runtime.)

Pipelining: W is DMAed in 4 quarter-slices (4KB per partition line --
2KB lines measured ~35% slower on the wire) split over both HWDGE rings
ahead of the 8 xs chunks, so the raw W tile a matmul needs is resident
long before its xs chunk lands; the PE pipelines chunk-by-chunk behind
the xs DMAs (which sustain the ~435GB/s SBUF-fabric ceiling) and
finishes right after the last byte.  Using raw W as the stationary
operand (instead of a DVE-reduced Wsum, as an earlier version did)
removes the W-reduce from the critical path entirely at zero PE cost:
matmul time scales only with the 320 streamed token columns.

Device layout trick: the PE contracts over the partition axis, but x in
DRAM is token-major.  The host pre-swizzles each core's x slice to
    xs[p, j*320 + t] = x2[t, c*8192 + p*64 + j]   (p in 0..128, j in 0..64)
so one matmul per j (lhsT = W slice (128,64), rhs = xs slice (128,320))
accumulates s64^T over 64 PSUM-accumulated matmuls.  W is pre-swizzled
to ws[p, (j*8+h)*8+d] = W[h*8+d, c*8192 + p*64 + j] so each j's lhsT is
a contiguous (128, 64) slice.  Matmul operands are float32r: same fp32
bytes, single-pass PE matmul at 1 cycle/row (plain fp32 is 4
cycles/row), costing ~1e-4 relative error.
"""

import numpy as np

import concourse.bass as bass
import concourse.mybir as mybir
import concourse.tile as tile
from concourse import bacc
from concourse.bass_utils import run_bass_kernel_spmd

B, NN, IN_DIM, OUT_DIM, HEADS = 32, 10, 65536, 64, 8
NCORES = 8
T = B * NN                 # 320 tokens
KS = IN_DIM // NCORES      # 8192 contraction slice per core
JW = KS // 128             # 64 j-steps per core
NCHUNK = 8                 # xs DMA chunks
JC = JW // NCHUNK          # j-steps per chunk
NWQ = 4                    # ws DMA quarters
WQC = JW * OUT_DIM // NWQ  # ws columns per quarter (1024 = 4KB/partition)
F32 = mybir.dt.float32
F32R = mybir.dt.float32r


def build_main():
    """Launch 1: per-core partial s64^T = (W k-slice) @ (x k-slice)^T."""
    nc = bacc.Bacc("TRN2", debug=False, num_devices=NCORES, target_bir_lowering=False)

    xs_d = nc.dram_tensor("xs", [128, JW * T], F32R, kind="ExternalInput").ap()
    ws_d = nc.dram_tensor("ws", [128, JW * OUT_DIM], F32R, kind="ExternalInput").ap()
    part_d = nc.dram_tensor("part", [OUT_DIM, T], F32, kind="ExternalOutput").ap()

    with tile.TileContext(nc) as tc:
        with (
            tc.tile_pool(name="xp", bufs=NCHUNK) as xp,
            tc.tile_pool(name="wp", bufs=1) as wp,
            tc.tile_pool(name="aux", bufs=1) as aux,
            tc.tile_pool(name="pp", bufs=1, space="PSUM") as pp,
        ):
            # ws quarters split over both HWDGE rings, ahead of the xs
            # chunks; quarter q covers js [16q, 16q+16) = xs chunks 2q, 2q+1
            wsts = []
            for q in range(NWQ):
                wst = wp.tile([128, WQC], F32R, name=f"wst{q}", tag="wst", bufs=NWQ)
                eng = nc.sync if q % 2 == 0 else nc.scalar
                eng.dma_start(wst[:], ws_d[:, q * WQC : (q + 1) * WQC])
                wsts.append(wst)

            # xs chunks alternate between the two HWDGE rings (SP and ACT)
            # so descriptor generation is not serialized on one engine
            psum_s = pp.tile([OUT_DIM, T], F32)
            for jc in range(NCHUNK):
                xt = xp.tile([128, JC * T], F32R, name=f"xt{jc}", tag="xt")
                eng = nc.sync if jc % 2 == 0 else nc.scalar
                eng.dma_start(
                    xt[:],
                    xs_d[:, jc * JC * T : (jc + 1) * JC * T],
                )
                for a in range(JC):
                    j = jc * JC + a
                    q, col = divmod(j * OUT_DIM, WQC)
                    nc.tensor.matmul(
                        psum_s[:],
                        wsts[q][:, col : col + OUT_DIM],
                        xt[:, a * T : (a + 1) * T],
                        start=(j == 0),
                        stop=(j == JW - 1),
                    )

            s_sbT = aux.tile([OUT_DIM, T], F32)
            nc.vector.tensor_copy(s_sbT[:], psum_s[:])
            nc.sync.dma_start(part_d[:], s_sbT[:])

    nc.compile()
    return nc


def build_fold():
    """Launch 2: core h folds head h's 64 partial rows, scales by adj^T, relu."""
    nc = bacc.Bacc("TRN2", debug=False, num_devices=NCORES, target_bir_lowering=False)

    # merged input: rows 0-63 = the 64 partial rows (core c, d) of this
    # core's head, rows 64-73 = adj^T (engine AP bases must be 0/32/64)
    fin_d = nc.dram_tensor("fin", [OUT_DIM + NN, T], F32R, kind="ExternalInput").ap()
    out_d = nc.dram_tensor("out", [NN, T], F32, kind="ExternalOutput").ap()

    with tile.TileContext(nc) as tc:
        with (
            tc.tile_pool(name="aux", bufs=1) as aux,
            tc.tile_pool(name="pp", bufs=1, space="PSUM") as pp,
        ):
            fin_sb = aux.tile([OUT_DIM + NN, T], F32R)
            nc.sync.dma_start(fin_sb[:], fin_d[:])
            # ones built on-device: no second input DMA
            ones_sb = aux.tile([OUT_DIM, NN], F32)
            nc.gpsimd.memset(ones_sb[:], 1.0)

            # ones-matmul: sums the 64 partial rows and replicates the sum
            # onto 10 PSUM partitions in one shot
            psum10 = pp.tile([NN, T], F32)
            nc.tensor.matmul(
                psum10[:],
                ones_sb[:].bitcast(F32R),
                fin_sb[:OUT_DIM, :],
                start=True,
                stop=True,
            )
            # relu(s)*adj == relu(s*adj) since adj >= 0; one fused DVE op
            res = aux.tile([NN, T], F32)
            nc.vector.scalar_tensor_tensor(
                out=res[:],
                in0=psum10[:],
                scalar=0.0,
                in1=fin_sb[OUT_DIM : OUT_DIM + NN, :].bitcast(F32),
                op0=mybir.AluOpType.max,
                op1=mybir.AluOpType.mult,
            )
            nc.sync.dma_start(out_d[:], res[:])

    nc.compile()
    return nc


def shard_inputs(x, adj, W):
    """Host-side sharding/layout (pure data movement, no math)."""
    x2 = np.ascontiguousarray(x, dtype=np.float32).reshape(T, IN_DIM)
    # xs[c][p, j*T + t] = x2[t, c*KS + p*JW + j]
    xv = x2.reshape(T, NCORES, 128, JW).transpose(1, 2, 3, 0)  # (c, p, j, t)
    xs_all = np.ascontiguousarray(xv).reshape(NCORES, 128, JW * T)
    # ws[c][p, (j*8+h)*8+d] = W[h*8+d, c*KS + p*JW + j]
    Wv = np.ascontiguousarray(W, dtype=np.float32).reshape(HEADS, 8, NCORES, 128, JW)
    wv = Wv.transpose(2, 3, 4, 0, 1)  # (c, p, j, h, d)
    ws_all = np.ascontiguousarray(wv).reshape(NCORES, 128, JW * OUT_DIM)
    return [{"xs": xs_all[c], "ws": ws_all[c]} for c in range(NCORES)]


_NC_MAIN = None
_NC_FOLD = None


def run(x, adj, W, trace=False, **kw):
    global _NC_MAIN, _NC_FOLD
    if _NC_MAIN is None:
        _NC_MAIN = build_main()
        _NC_FOLD = build_fold()

    res1 = run_bass_kernel_spmd(
        _NC_MAIN, shard_inputs(x, adj, W), core_ids=list(range(NCORES)),
        trace=trace, **kw
    )
    # host gather/scatter of the 80KB partials: core h gets rows h*8+d of
    # every core's partial s64^T (pure data movement)
    parts = np.stack([res1.results[c]["part"] for c in range(NCORES)])  # (c, hd, t)
    adjt = np.asarray(adj, dtype=np.float32).reshape(T, NN).T
    in_maps2 = []
    for h in range(HEADS):
        fin = np.empty((OUT_DIM + NN, T), dtype=np.float32)
        # fin[c*8+d] = parts[c, h*8+d]
        fin[:OUT_DIM] = parts[:, h * 8 : (h + 1) * 8, :].reshape(OUT_DIM, T)
        fin[OUT_DIM:] = adjt
        in_maps2.append({"fin": fin})
    res2 = run_bass_kernel_spmd(
        _NC_FOLD, in_maps2, core_ids=list(range(NCORES)), trace=trace, **kw
    )

    full = np.empty((T, HEADS * NN), dtype=np.float32)
    for h in range(HEADS):
        full[:, h * NN : (h + 1) * NN] = res2.results[h]["out"].T
    return full.reshape(B, NN, HEADS * NN), (res1, res2)


def kernel(x, adj, W):
    out, _ = run(x, adj, W)
    return out


# revision 8
# speedup vs baseline: 1.0276x; 1.0276x over previous
"""Trainium2 Bass kernel for nn_GATLayer_58291296141986.

Math: the reference computes
    xt = (x @ W.T).reshape(B, N, H, D)            # B=32, N=10, H=8, D=8
    out[b,n,h,m] = relu(sum_k xt[b,n,h,k] * adj[b,n,m])
adj does not depend on k, so sum_k xt[b,n,h,k] = x[b,n,:] @ Wsum[h,:]
with Wsum[h] = sum_d W[h*8+d].  The whole problem collapses to
    s = x2 @ Wsum.T        # (320, 65536) @ (65536, 8)
    out[t, h*10+m] = relu(s[t,h] * adj[t,m])
which is memory-bound on reading x (84MB) + W (17MB).

Sharding: tensor-parallel over in_dim (k).  Each of the 8 cores reads a
disjoint 8192-wide k-slice of x (10.5MB) and W (2MB) and accumulates a
partial s64^T = W_slice @ x_slice^T of shape (64, 320) -- every input
byte is read exactly once across the chip (~12.6MB/core, the memory
roofline).  A tail selector-matmul folds the d dimension (rows h*8+d ->
head h), so each core outputs an (8, 320) partial of s.  The cross-core
reduction of those 10KB partials happens in a second, tiny SPMD launch:
the host hands core h the 8 partial rows of head h (pure data movement),
a ones-matmul sums them and replicates the result onto 10 PSUM
partitions, and one fused scalar_tensor_tensor computes relu(s)*adj^T
(valid since adj >= 0, so relu(s*adj) = relu(s)*adj).  Core h thus
produces the 10 output columns of head h for all 320 tokens and the
host concatenates the 8 head slices.  (A single-launch variant with an
on-device AllToAll was measured ~30us slower: the collectives
firmware's entry barrier alone costs ~60us on this runtime; and a
minimal-kernel probe measured ~12.6us of fixed NRT pre/postamble per
launch, so the fold launch is already near its floor.)

Pipelining: W is DMAed in 4 quarter-slices (4KB per partition line --
2KB lines measured ~35% slower on the wire) split over both HWDGE rings
ahead of the 8 xs chunks; raw W slices are the stationary operand (no
DVE reduce anywhere near the critical path), and the PE pipelines
chunk-by-chunk behind the xs DMAs, which sustain the ~435GB/s
SBUF-fabric ceiling.

HAM warming: the PE's activity monitor keeps the array at 1.2GHz unless
it has been busy ~3.4us continuously, and chunked matmul bursts (2us of
work every ~3.2us) never warm it -- profiled cold matmuls at 468ns vs
309ns warm made the PE fall behind the DMA stream and trail ~5us past
the last byte.  Dummy filler matmuls (into a scratch PSUM bank, on the
already-resident W tile) prime the array before the first chunk and pad
the idle between chunks, keeping the clock at 2.4GHz throughout.

Device layout trick: the PE contracts over the partition axis, but x in
DRAM is token-major.  The host pre-swizzles each core's x slice to
    xs[p, j*320 + t] = x2[t, c*8192 + p*64 + j]   (p in 0..128, j in 0..64)
so one matmul per j (lhsT = W slice (128,64), rhs = xs slice (128,320))
accumulates s64^T over 64 PSUM-accumulated matmuls.  W is pre-swizzled
to ws[p, (j*8+h)*8+d] = W[h*8+d, c*8192 + p*64 + j] so each j's lhsT is
a contiguous (128, 64) slice.  Matmul operands are float32r: same fp32
bytes, single-pass PE matmul at 1 cycle/row (plain fp32 is 4
cycles/row), costing ~1e-4 relative error.
"""

import numpy as np

import concourse.bass as bass
import concourse.mybir as mybir
import concourse.tile as tile
from concourse import bacc
from concourse.bass_utils import run_bass_kernel_spmd

B, NN, IN_DIM, OUT_DIM, HEADS = 32, 10, 65536, 64, 8
NCORES = 8
T = B * NN                 # 320 tokens
KS = IN_DIM // NCORES      # 8192 contraction slice per core
JW = KS // 128             # 64 j-steps per core
NCHUNK = 8                 # xs DMA chunks
JC = JW // NCHUNK          # j-steps per chunk
NWQ = 4                    # ws DMA quarters
WQC = JW * OUT_DIM // NWQ  # ws columns per quarter (1024 = 4KB/partition)
PRIME_FILL = 60            # HAM-priming filler matmuls before chunk 0
CHUNK_FILL = 8             # idle-padding fillers after each chunk burst
F32 = mybir.dt.float32
F32R = mybir.dt.float32r


def build_main():
    """Launch 1: per-core partial s^T = fold_d((W k-slice) @ (x k-slice)^T)."""
    nc = bacc.Bacc("TRN2", debug=False, num_devices=NCORES, target_bir_lowering=False)

    xs_d = nc.dram_tensor("xs", [128, JW * T], F32R, kind="ExternalInput").ap()
    ws_d = nc.dram_tensor("ws", [128, JW * OUT_DIM], F32R, kind="ExternalInput").ap()
    sel_d = nc.dram_tensor("sel", [OUT_DIM, HEADS], F32R, kind="ExternalInput").ap()
    part_d = nc.dram_tensor("part", [HEADS, T], F32, kind="ExternalOutput").ap()

    with tile.TileContext(nc) as tc:
        with (
            tc.tile_pool(name="xp", bufs=NCHUNK) as xp,
            tc.tile_pool(name="wp", bufs=1) as wp,
            tc.tile_pool(name="aux", bufs=1) as aux,
            tc.tile_pool(name="pp", bufs=1, space="PSUM") as pp,
        ):
            # ws quarters split over both HWDGE rings, ahead of the xs
            # chunks; quarter q covers js [16q, 16q+16) = xs chunks 2q, 2q+1
            wsts = []
            for q in range(NWQ):
                wst = wp.tile([128, WQC], F32R, name=f"wst{q}", tag="wst", bufs=NWQ)
                eng = nc.sync if q % 2 == 0 else nc.scalar
                eng.dma_start(wst[:], ws_d[:, q * WQC : (q + 1) * WQC])
                wsts.append(wst)
            sel_sb = aux.tile([OUT_DIM, HEADS], F32R)
            nc.scalar.dma_start(sel_sb[:], sel_d[:])

            psum_s = pp.tile([OUT_DIM, T], F32, name="psum_s")
            psum_junk = pp.tile([OUT_DIM, T], F32, name="psum_junk")

            def filler(n):
                # garbage matmuls on the resident ws quarter: keep the PE
                # busy so HAM holds the array at 2.4GHz (values unused)
                for _ in range(n):
                    nc.tensor.matmul(
                        psum_junk[:, :OUT_DIM],
                        wsts[0][:, :OUT_DIM],
                        wsts[0][:, :OUT_DIM],
                        start=True,
                        stop=True,
                    )

            filler(PRIME_FILL)

            # xs chunks alternate between the two HWDGE rings (SP and ACT)
            # so descriptor generation is not serialized on one engine
            for jc in range(NCHUNK):
                xt = xp.tile([128, JC * T], F32R, name=f"xt{jc}", tag="xt")
                eng = nc.sync if jc % 2 == 0 else nc.scalar
                eng.dma_start(
                    xt[:],
                    xs_d[:, jc * JC * T : (jc + 1) * JC * T],
                )
                for a in range(JC):
                    j = jc * JC + a
                    q, col = divmod(j * OUT_DIM, WQC)
                    nc.tensor.matmul(
                        psum_s[:],
                        wsts[q][:, col : col + OUT_DIM],
                        xt[:, a * T : (a + 1) * T],
                        start=(j == 0),
                        stop=(j == JW - 1),
                    )
                if jc < NCHUNK - 1:
                    filler(CHUNK_FILL)

            # tail: d-fold via selector matmul, then the 10KB partial out
            s64_sb = aux.tile([OUT_DIM, T], F32R)
            with nc.allow_low_precision(
                reason="f32r rounding of s64 is the intended matmul precision"
            ):
                nc.vector.tensor_copy(s64_sb[:], psum_s[:])
            psum8 = pp.tile([HEADS, T], F32, name="psum8")
            nc.tensor.matmul(psum8[:], sel_sb[:], s64_sb[:], start=True, stop=True)
            s_sbT = aux.tile([HEADS, T], F32)
            nc.vector.tensor_copy(s_sbT[:], psum8[:])
            nc.sync.dma_start(part_d[:], s_sbT[:])

    nc.compile()
    return nc


def build_fold():
    """Launch 2: core h folds head h's 8 partials, scales by adj^T, relu."""
    nc = bacc.Bacc("TRN2", debug=False, num_devices=NCORES, target_bir_lowering=False)

    # merged input: rows 0-7 = the 8 partials of this core's head,
    # rows 32-41 = adj^T (bases 0/32: engine APs only support 0/32/64)
    fin_d = nc.dram_tensor("fin", [32 + NN, T], F32R, kind="ExternalInput").ap()
    out_d = nc.dram_tensor("out", [NN, T], F32, kind="ExternalOutput").ap()

    with tile.TileContext(nc) as tc:
        with (
            tc.tile_pool(name="aux", bufs=1) as aux,
            tc.tile_pool(name="pp", bufs=1, space="PSUM") as pp,
        ):
            fin_sb = aux.tile([32 + NN, T], F32R)
            nc.sync.dma_start(fin_sb[:], fin_d[:])
            # ones built on-device: no second input DMA
            ones_sb = aux.tile([NCORES, NN], F32)
            nc.gpsimd.memset(ones_sb[:], 1.0)

            # ones-matmul: sums the 8 partial rows and replicates the sum
            # onto 10 PSUM partitions in one shot
            psum10 = pp.tile([NN, T], F32)
            nc.tensor.matmul(
                psum10[:],
                ones_sb[:].bitcast(F32R),
                fin_sb[:NCORES, :],
                start=True,
                stop=True,
            )
            # relu(s)*adj == relu(s*adj) since adj >= 0; one fused DVE op
            res = aux.tile([NN, T], F32)
            nc.vector.scalar_tensor_tensor(
                out=res[:],
                in0=psum10[:],
                scalar=0.0,
                in1=fin_sb[32 : 32 + NN, :].bitcast(F32),
                op0=mybir.AluOpType.max,
                op1=mybir.AluOpType.mult,
            )
            nc.sync.dma_start(out_d[:], res[:])

    nc.compile()
    return nc


def shard_inputs(x, adj, W):
    """Host-side sharding/layout (pure data movement, no math)."""
    x2 = np.ascontiguousarray(x, dtype=np.float32).reshape(T, IN_DIM)
    # xs[c][p, j*T + t] = x2[t, c*KS + p*JW + j]
    xv = x2.reshape(T, NCORES, 128, JW).transpose(1, 2, 3, 0)  # (c, p, j, t)
    xs_all = np.ascontiguousarray(xv).reshape(NCORES, 128, JW * T)
    # ws[c][p, (j*8+h)*8+d] = W[h*8+d, c*KS + p*JW + j]
    Wv = np.ascontiguousarray(W, dtype=np.float32).reshape(HEADS, 8, NCORES, 128, JW)
    wv = Wv.transpose(2, 3, 4, 0, 1)  # (c, p, j, h, d)
    ws_all = np.ascontiguousarray(wv).reshape(NCORES, 128, JW * OUT_DIM)
    # selector: S[h*8+d, h'] = 1 iff h == h'  (d-fold on the PE)
    sel = np.kron(np.eye(HEADS, dtype=np.float32), np.ones((8, 1), dtype=np.float32))
    return [{"xs": xs_all[c], "ws": ws_all[c], "sel": sel} for c in range(NCORES)]


_NC_MAIN = None
_NC_FOLD = None


def run(x, adj, W, trace=False, **kw):
    global _NC_MAIN, _NC_FOLD
    if _NC_MAIN is None:
        _NC_MAIN = build_main()
        _NC_FOLD = build_fold()

    res1 = run_bass_kernel_spmd(
        _NC_MAIN, shard_inputs(x, adj, W), core_ids=list(range(NCORES)),
        trace=trace, **kw
    )
    # host gather/scatter of the 10KB partials: core h gets row h of every
    # core's partial s^T (pure data movement)
    parts = np.stack([res1.results[c]["part"] for c in range(NCORES)])  # (c, h, t)
    adjt = np.asarray(adj, dtype=np.float32).reshape(T, NN).T
    in_maps2 = []
    for h in range(HEADS):
        fin = np.zeros((32 + NN, T), dtype=np.float32)
        fin[:NCORES] = parts[:, h, :]
        fin[32:] = adjt
        in_maps2.append({"fin": fin})
    res2 = run_bass_kernel_spmd(
        _NC_FOLD, in_maps2, core_ids=list(range(NCORES)), trace=trace, **kw
    )

    full = np.empty((T, HEADS * NN), dtype=np.float32)
    for h in range(HEADS):
        full[:, h * NN : (h + 1) * NN] = res2.results[h]["out"].T
    return full.reshape(B, NN, HEADS * NN), (res1, res2)


def kernel(x, adj, W):
    out, _ = run(x, adj, W)
    return out
